# revision 10
# baseline (speedup 1.0000x reference)
"""Bass/Trainium2 kernel for nn_Attention_84688165142614 (additive attention).

Computes, for full inputs (B=32, S=2048, EH=512, DH=512):
    enc    = enc_output.transpose(1, 0, 2)                  # [B, S, 2EH]
    energy = tanh(enc @ w_enc + (h @ w_dec) + attn_b)       # [B, S, DH]
    att    = energy @ v_w                                   # [B, S]
    att    = where(mask == 0, -1e10, att)
    out    = softmax(att, axis=1)

Strategy: data-parallel over batch across 8 NeuronCores (4 batches/core),
plus mask-sparsity compaction. The mask is ~50% zeros and masked positions
produce exactly 0 in the reference output (exp(-1e10) underflows in f32),
so the host keeps only unmasked source positions per batch (gather),
pads each batch to a fixed multiple of 128 columns, transposes the kept
enc columns feature-major and pre-casts to bf16. The device computes
energies/logits/softmax only for the compacted columns (pads are killed
with a -1e10 additive mask row), and the host scatters the compacted
probabilities back into a zero [B, S] output.

Device structure: the big matmul runs in bf16 with fp32 PSUM accumulation.
The enc DRAM shard is laid out group-major — exactly the order the PE
consumes it (per batch, per PSUM group of 4 s-tiles, ec-major inside) —
so each (batch, group) is one contiguous DMA into its own SBUF tile,
rotated across the three DMA lanes (sync/scalar HWDGE + gpsimd SWDGE).
Warmup matmuls on memset data keep the PE busy (and the HAM clock
un-throttled) during the initial fill. tanh output and v are bf16, which
gives affine_mul_reduce the 2x DVE mode. Softmax skips max-subtraction
(logits are bounded by sum|v| ~ 8); all logits are shifted by -8*ln2
(softmax is shift-invariant) so exp sums fit fp16, letting the
partition-sum broadcast matmul run 1-pass fp16 instead of 2-pass fp32.
"""

import numpy as np
from contextlib import ExitStack

import concourse.bass as bass
import concourse.tile as tile
from concourse import bacc, mybir
from concourse.bass_utils import run_bass_kernel_spmd

# Problem shape (hardcoded; kernel.py must be self-contained).
B, S, E2, DH = 32, 2048, 1024, 512
N_CORES = 8
BC = B // N_CORES        # batches per core = 4
P = 128                  # SBUF partitions
EC = E2 // P             # enc-feature chunks = 8
D = DH                   # 512
KC = DH // P             # dec-feature chunks = 4
NT_DEFAULT = 9           # compacted s-tiles per batch (1152 cols, ~5.7 sigma)

f32 = mybir.dt.float32
bf16 = mybir.dt.bfloat16
fp16 = mybir.dt.float16
AF = mybir.ActivationFunctionType
ALU = mybir.AluOpType

NEG_BIG = -1.0e10
SHIFT = 8.0 * 0.6931471805599453  # logit shift so exp sums fit fp16

_NC_CACHE = {}


def _group_sizes(nt):
    sizes = [4] * (nt // 4)
    if nt % 4:
        sizes.append(nt % 4)
    return sizes


def _emit(ctx, tc, nc, nt, enc_t, hwdec, madd_in, w_enc, sel_in, bv_in, out):
    W = nt * P
    sizes = _group_sizes(nt)
    starts = [sum(sizes[:i]) for i in range(len(sizes))]
    goffs = [P * sum(sizes[:i]) * EC for i in range(len(sizes))]  # col offset of group

    const = ctx.enter_context(tc.tile_pool(name="const", bufs=1))
    spsum = ctx.enter_context(tc.tile_pool(name="spsum", bufs=1, space="PSUM"))
    mpsum = ctx.enter_context(tc.tile_pool(name="mpsum", bufs=7, space="PSUM"))
    encp = ctx.enter_context(tc.tile_pool(name="encp", bufs=BC * len(sizes)))
    tmpp = ctx.enter_context(tc.tile_pool(name="tmpp", bufs=3))
    thp = ctx.enter_context(tc.tile_pool(name="thp", bufs=3))
    scrp = ctx.enter_context(tc.tile_pool(name="scrp", bufs=2))
    attp = ctx.enter_context(tc.tile_pool(name="attp", bufs=2))
    epip = ctx.enter_context(tc.tile_pool(name="epip", bufs=10))

    # ---- warmup source tiles (no DMA deps): keep the PE busy during fill ----
    ones16 = const.tile([P, P], fp16)
    nc.vector.memset(ones16[:], 1.0)
    wsrc = const.tile([P, D], fp16)
    nc.vector.memset(wsrc[:], 0.001)
    ones_row = const.tile([1, P], bf16)
    nc.vector.memset(ones_row[:], 1.0)

    # ---- DMA lane schedules -------------------------------------------------
    # Per-(batch, group) enc tile [P, EC*Wg]; DRAM is laid out in the same
    # group-major order, so every group is one contiguous transfer.
    gtiles = {}
    for b in range(BC):
        for sg, gsz in enumerate(sizes):
            gtiles[(b, sg)] = encp.tile(
                [P, EC * gsz * P], bf16, tag="enc", name=f"enc_{b}_{sg}"
            )

    wq = const.tile([P, EC * D], bf16)
    hwdec_sb = const.tile([P, KC * BC + KC * D], bf16)
    madd_sb = const.tile([P, BC * nt], f32)
    sel_sb = const.tile([BC, BC * P], bf16)
    bv_sb = const.tile([1, 2 * D], bf16)

    # All enc/wq transfers go on the single sync HWDGE ring, in exact
    # consumption order: a single ring drains FIFO across all 16 SDMA
    # engines at near-full HBM bandwidth, so the first-needed transfer is
    # never stuck behind fair-share round-robin with later ones. The
    # scalar ring carries only the tiny consts (and output stores), so
    # ACTIVATEs never queue behind a waiting DMA issue; gpsimd (SWDGE)
    # is not used at all.
    g00 = gtiles[(0, 0)]
    hw = EC * sizes[0] * P // 2
    nc.sync.dma_start(out=wq[:, 0 : 2 * D], in_=w_enc[:, 0 : 2 * D])
    nc.sync.dma_start(out=g00[:, 0:hw], in_=enc_t[0, :, 0:hw])
    nc.sync.dma_start(out=wq[:, 2 * D : 4 * D], in_=w_enc[:, 2 * D : 4 * D])
    nc.sync.dma_start(out=g00[:, hw : 2 * hw], in_=enc_t[0, :, hw : 2 * hw])
    nc.sync.dma_start(out=wq[:, 4 * D : 8 * D], in_=w_enc[:, 4 * D : 8 * D])

    # small consts on scalar lane
    nc.scalar.dma_start(out=hwdec_sb[:], in_=hwdec[:])
    nc.scalar.dma_start(out=madd_sb[:], in_=madd_in[:])
    nc.scalar.dma_start(out=sel_sb[:], in_=sel_in[:])
    nc.scalar.dma_start(out=bv_sb[:], in_=bv_in[:])

    # remaining groups on the sync ring in consumption order
    rest = [(b, sg) for b in range(BC) for sg in range(len(sizes))][1:]
    for b, sg in rest:
        gsz = sizes[sg]
        lo = goffs[sg]
        hi = lo + EC * gsz * P
        nc.sync.dma_start(out=gtiles[(b, sg)][:], in_=enc_t[b, :, lo:hi])

    # ---- PE warmup: 8 matmuls on memset data (~3.4us cold, HAM -> 8/8) ----
    wps = spsum.tile([P, D], f32, tag="sp", name="warm")
    for i in range(8):
        nc.tensor.matmul(wps[:], lhsT=ones16[:], rhs=wsrc[:], start=True, stop=True)

    # ---- dec[b, :] = h[b] @ w_dec + attn_b; broadcasts ----
    HW0 = KC * BC  # offset of w_dec columns inside hwdec_sb
    dec_ps = spsum.tile([BC, D], f32, tag="sp")
    for kc in range(KC):
        nc.tensor.matmul(
            dec_ps[:],
            lhsT=hwdec_sb[:, kc * BC : (kc + 1) * BC],
            rhs=hwdec_sb[:, HW0 + kc * D : HW0 + (kc + 1) * D],
            start=(kc == 0),
            stop=False,
        )
    nc.tensor.matmul(
        dec_ps[:], lhsT=ones_row[:, 0:BC], rhs=bv_sb[:, 0:D], start=False, stop=True
    )
    dec_rows = const.tile([BC, D], bf16)
    nc.vector.tensor_copy(dec_rows[:], dec_ps[:])

    dec_bc = const.tile([P, BC * D], f32)
    for b in range(BC):
        ps = spsum.tile([P, D], f32, tag="sp", name=f"decb_{b}")
        nc.tensor.matmul(
            ps[:], lhsT=sel_sb[:, b * P : (b + 1) * P], rhs=dec_rows[:],
            start=True, stop=True,
        )
        nc.vector.tensor_copy(dec_bc[:, b * D : (b + 1) * D], ps[:])
    v_ps = spsum.tile([P, D], f32, tag="sp")
    nc.tensor.matmul(
        v_ps[:], lhsT=ones_row[:], rhs=bv_sb[:, D : 2 * D], start=True, stop=True
    )
    v_sb = const.tile([P, D], bf16)
    nc.vector.tensor_copy(v_sb[:], v_ps[:])

    # ---- main loop ----
    for b in range(BC):
        # For the last batch, fold the dec add into the matmul accumulation
        # (a K=4 matmul against the sel column block) so tanh reads PSUM
        # directly and the vector-engine backlog drains before the kernel
        # tail instead of after it.
        dec_in_mm = b == BC - 1
        att = attp.tile([P, nt], f32, tag="att", name=f"att_{b}")
        for sg, gsz in enumerate(sizes):
            gt = gtiles[(b, sg)]
            Wg = gsz * P
            psums = [
                mpsum.tile([P, D], f32, tag="mm", name=f"mm_{b}_{sg}_{j}")
                for j in range(gsz)
            ]
            for ec in range(EC):
                for j in range(gsz):
                    nc.tensor.matmul(
                        psums[j][:],
                        lhsT=gt[:, ec * Wg + j * P : ec * Wg + (j + 1) * P],
                        rhs=wq[:, ec * D : (ec + 1) * D],
                        start=(ec == 0),
                        stop=(ec == EC - 1) and not dec_in_mm,
                    )
            if dec_in_mm:
                for j in range(gsz):
                    nc.tensor.matmul(
                        psums[j][:],
                        lhsT=sel_sb[:, b * P : (b + 1) * P],
                        rhs=dec_rows[:],
                        start=False,
                        stop=True,
                    )
            for j in range(gsz):
                st = starts[sg] + j
                th = thp.tile([P, D], bf16, tag="th")
                if dec_in_mm:
                    nc.scalar.activation(th[:], psums[j][:], AF.Tanh)
                else:
                    t_sb = tmpp.tile([P, D], f32, tag="tmp")
                    nc.vector.tensor_add(
                        t_sb[:], psums[j][:], dec_bc[:, b * D : (b + 1) * D]
                    )
                    nc.scalar.activation(th[:], t_sb[:], AF.Tanh)
                scr = scrp.tile([P, D], bf16, tag="scr")
                nc.vector.affine_mul_reduce(
                    out=scr[:],
                    accum_out=att[:, st : st + 1],
                    in0=th[:],
                    in1=v_sb[:],
                    scale=1.0,
                    bias=0.0,
                )

        # ---- epilogue: mask+shift, exp, fp16 partition-sum bcast, scale ----
        attm = epip.tile([P, nt], f32, tag="attm", name=f"attm_{b}")
        nc.vector.tensor_add(attm[:], att[:], madd_sb[:, b * nt : (b + 1) * nt])
        expt = epip.tile([P, nt], f32, tag="expt", name=f"expt_{b}")
        partial = epip.tile([P, 1], f32, tag="partial", name=f"psum_{b}")
        nc.scalar.activation(expt[:], attm[:], AF.Exp)
        nc.vector.tensor_reduce(partial[:], expt[:], mybir.AxisListType.X, ALU.add)
        p16 = epip.tile([P, 1], fp16, tag="p16", name=f"p16_{b}")
        nc.vector.tensor_copy(p16[:], partial[:])
        tot_ps = spsum.tile([P, 1], f32, tag="sp", name=f"tot_{b}")
        nc.tensor.matmul(
            tot_ps[:], lhsT=ones16[:], rhs=p16[:], start=True, stop=True
        )
        r_pp = epip.tile([P, 1], f32, tag="rpp", name=f"rpp_{b}")
        nc.vector.reciprocal(r_pp[:], tot_ps[:])
        out_sb = epip.tile([P, nt], f32, tag="outsb", name=f"osb_{b}")
        nc.vector.tensor_scalar_mul(out_sb[:], expt[:], r_pp[:])
        nc.scalar.dma_start(out=out[b], in_=out_sb[:])


def build_nc(nt):
    if nt in _NC_CACHE:
        return _NC_CACHE[nt]
    W = nt * P
    nc = bacc.Bacc("TRN2", target_bir_lowering=False, debug=False)
    enc_t = nc.dram_tensor("enc_t", [BC, P, EC * W], bf16, kind="ExternalInput").ap()
    hwdec = nc.dram_tensor(
        "hwdec", [P, KC * BC + KC * D], bf16, kind="ExternalInput"
    ).ap()
    madd = nc.dram_tensor("madd", [P, BC * nt], f32, kind="ExternalInput").ap()
    w_enc = nc.dram_tensor("w_enc", [P, EC * D], bf16, kind="ExternalInput").ap()
    sel_in = nc.dram_tensor("sel_in", [BC, BC * P], bf16, kind="ExternalInput").ap()
    bv = nc.dram_tensor("bv", [1, 2 * D], bf16, kind="ExternalInput").ap()
    out = nc.dram_tensor("out", [BC, P, nt], f32, kind="ExternalOutput").ap()

    with tile.TileContext(nc) as tc:
        with ExitStack() as ctx:
            _emit(ctx, tc, nc, nt, enc_t, hwdec, madd, w_enc, sel_in, bv, out)
    nc.compile()
    _NC_CACHE[nt] = nc
    return nc


def shard_inputs(inputs, nt):
    import ml_dtypes

    W = nt * P
    sizes = _group_sizes(nt)
    h = np.asarray(inputs["h"], dtype=np.float32)
    enc = np.asarray(inputs["enc_output"], dtype=np.float32)
    mask = np.asarray(inputs["mask"], dtype=np.int32)
    attn_w = np.asarray(inputs["attn_w"], dtype=np.float32)
    attn_b = np.asarray(inputs["attn_b"], dtype=np.float32)
    v_w = np.asarray(inputs["v_w"], dtype=np.float32)

    # w_dec [DH, D] -> [P, KC*D] with free index (kc, d)
    w_dec = np.ascontiguousarray(
        attn_w[:DH].reshape(KC, P, D).transpose(1, 0, 2).reshape(P, KC * D)
    )
    # w_enc [E2, D] -> [P, EC*D] with free index (ec, d), pre-cast to bf16
    w_enc = np.ascontiguousarray(
        attn_w[DH:].reshape(EC, P, D).transpose(1, 0, 2).reshape(P, EC * D)
    ).astype(ml_dtypes.bfloat16)

    sel_np = np.zeros((BC, BC * P), dtype=ml_dtypes.bfloat16)
    for b in range(BC):
        sel_np[b, b * P : (b + 1) * P] = 1.0
    bv = np.concatenate([attn_b, v_w]).reshape(1, 2 * D).astype(ml_dtypes.bfloat16)

    kept = [np.nonzero(mask[gb])[0] for gb in range(B)]

    in_maps = []
    for c in range(N_CORES):
        enc_c = np.zeros((BC, P, EC * W), dtype=ml_dtypes.bfloat16)
        madd = np.full((P, BC * nt), NEG_BIG, dtype=np.float32)
        h_t = (
            h[BC * c : BC * (c + 1)]
            .T.reshape(KC, P, BC)
            .transpose(1, 0, 2)
            .reshape(P, KC * BC)
        )
        hwdec = np.concatenate([h_t, w_dec], axis=1).astype(ml_dtypes.bfloat16)
        for b in range(BC):
            gb = BC * c + b
            idx = kept[gb]
            n = len(idx)
            # kept enc columns, feature-major, padded: [EC, P, W]
            padded = np.zeros((EC, P, W), dtype=ml_dtypes.bfloat16)
            cols = enc[idx, gb, :].T.astype(ml_dtypes.bfloat16)
            padded[:, :, :n] = cols.reshape(EC, P, n)
            # group-major column order: per group sg, ec-major block
            off = 0
            parts = []
            for gsz in sizes:
                blk = padded[:, :, off : off + gsz * P]      # [EC, P, Wg]
                parts.append(blk.transpose(1, 0, 2).reshape(P, EC * gsz * P))
                off += gsz * P
            enc_c[b] = np.concatenate(parts, axis=1)
            # compact additive mask: -shift for real columns, -1e10 for pads
            flat = madd.reshape(P, BC, nt)
            m = np.full(W, -SHIFT, dtype=np.float32)
            m[n:] = NEG_BIG
            flat[:, b, :] = m.reshape(nt, P).T
        in_maps.append(
            dict(
                enc_t=enc_c, hwdec=hwdec, madd=madd, w_enc=w_enc,
                sel_in=sel_np, bv=bv,
            )
        )
    return in_maps, kept


def run(inputs, trace=False):
    mask = np.asarray(inputs["mask"], dtype=np.int32)
    counts = mask.sum(axis=1)
    nt = max(NT_DEFAULT, int(np.ceil(counts.max() / P)))
    nc = build_nc(nt)
    in_maps, kept = shard_inputs(inputs, nt)
    res = run_bass_kernel_spmd(nc, in_maps, list(range(N_CORES)), trace=trace)
    out_full = np.zeros((B, S), dtype=np.float32)
    for c in range(N_CORES):
        vals = res.results[c]["out"].reshape(BC, P, nt)
        for b in range(BC):
            gb = BC * c + b
            idx = kept[gb]
            flat = vals[b].T.reshape(nt * P)
            out_full[gb, idx] = flat[: len(idx)]
    return out_full, res


def kernel(**inputs) -> np.ndarray:
    out, _ = run(inputs, trace=False)
    return out
